# revision 59
# baseline (speedup 1.0000x reference)
"""AdaptiveTokenMixer Trainium2 kernel (8 NeuronCores, pure data parallel).

Per-core algorithm (one batch element per core), pipelined over 2 chunks
(22+12 position-blocks of BLK=121 outputs; BLK+K-1 = 128 fills the PE
contraction window exactly):
  1. alpha stage: delta-time / valid-mask windows are host-packed into
     [128, (block, tap)] im2col layout (pure layout, no host arithmetic),
     so the device softmax starts immediately after one constant DMA:
     cv = vw*vw0 mask products, lgn = (dtw - BIG)*cv, masked temporal-decay
     softmax over K=8 offsets via a min-reduce (negated-exponent form),
     blended as au = (e + s*c)*cv (scale-invariant rewrite avoids one
     reciprocal), renormalized -> af bf16. Chunk A runs its elementwise
     ops on vector, chunk B on gpsimd (free-axis reduces are vector-only).
  2. W stage (per chunk): af chunk written to a DRAM scratch with a SKEWED
     access pattern (banded W^T[m, k] = alpha[n0+m, k-m], m-major 128x128
     tiles over a zeros-initialized buffer); loaded back with an
     XBAR-transposing DMA into W[k, m] orientation. The skews are issued
     on the SP ring (sync) and the XBAR loads on the Act ring (scalar):
     a same-ring skew->XBAR pair loses its semaphore wait to the ring-FIFO
     assumption and the XBAR read races the skew's multi-engine descriptor
     drain; concurrent DMA_TRANSPOSEs on the two rings also corrupt each
     other (single shared XBAR unit), so both loads stay on Act.
  3. Mix (per block): out[m, :] = sum_k W[k, m] * x[n0+k, :] -- one
     128x128 @ 128x256 bf16 matmul per block (PSUM f32), two blocks per
     PSUM bank.
  4. Evict PSUM -> SBUF bf16 (paired, alternating vector/scalar), group
     stores (8,8,8,8,3) overlapping later matmuls.

Self-contained: hardcodes shapes for B=8, N=4096, d=256, K=8.
"""
import numpy as np
import ml_dtypes

import concourse.bass as bass
import concourse.bacc as bacc
import concourse.mybir as mybir
from concourse import tile
from concourse.bass_utils import run_bass_kernel_spmd

B, N, D, K = 8, 4096, 256, 8
BLK = 121                      # output positions per block (BLK+K-1=128)
NB = (N + BLK - 1) // BLK      # 35 blocks -> covers 4200 positions
NOUT = NB * BLK                # 4200 rows in padded device output
NPAD = 4224                    # padded input length (>= 33*121 + 135)
KW = 128                       # k-window (contraction) per block
WBLK = KW * KW                 # W scratch elements per block
F = K * NB                     # alpha free size (b-major, p-minor)
BIG = 1024.0
CA = 24                        # chunk A blocks (vector chain)
CB = NB - CA                   # chunk B blocks (gpsimd chain)

_CACHE = {}


def _build():
    nc = bacc.Bacc("TRN2", target_bir_lowering=False, debug=False,
                   num_devices=B)
    f32 = mybir.dt.float32
    bf16 = mybir.dt.bfloat16

    x_t = nc.dram_tensor("x", [NPAD, D], bf16, kind="ExternalInput")
    # cta: dt windows [128,(b,p)] f32 (cols 0:280) | bwsm (280:288)
    cta_t = nc.dram_tensor("cta", [128, 288], f32, kind="ExternalInput")
    # ctb: vf windows [128,(b,p)] bf16 (cols 0:280) | vf0 [128, NB] (280:315)
    ctb_t = nc.dram_tensor("ctb", [128, 315], bf16, kind="ExternalInput")
    wza_t = nc.dram_tensor("wza", [CA * WBLK], bf16, kind="ExternalInput")
    wzb_t = nc.dram_tensor("wzb", [CB * WBLK], bf16, kind="ExternalInput")
    out_t = nc.dram_tensor("out", [NOUT, D], bf16, kind="ExternalOutput")

    def pb_h(a, c0, cn):  # [128, b, p] view of a (b,p)-flat AP, block range
        return bass.AP(a.tensor, a.offset + c0 * K,
                       [a.ap[0], [K, cn], [1, K]])

    def exp_nb_h(a, b0, nb):  # block-range slice of a per-block broadcast
        return bass.AP(a.tensor, a.offset + b0 * a.ap[1][0],
                       [a.ap[0], [a.ap[1][0], nb], [0, K]])

    def exp_k_h(a, nb):       # block-range slice of a per-tap broadcast
        return bass.AP(a.tensor, a.offset, [a.ap[0], [0, nb], [1, K]])

    with tile.TileContext(nc) as tc:
        with tc.tile_pool(name="alph", bufs=1) as apool, \
             tc.tile_pool(name="outg", bufs=5) as opool, \
             tc.tile_pool(name="big", bufs=1) as bpool, \
             tc.tile_pool(name="psB", bufs=4, space="PSUM") as psB:

            # ---- engine warm-up (absorbs first-op ucode penalties) ----
            jv = apool.tile([2, 8], f32)
            nc.vector.memset(jv[:], 0.0)
            nc.vector.tensor_tensor(jv[:], jv[:], jv[:],
                                    mybir.AluOpType.mult)
            jg = apool.tile([2, 8], f32)
            nc.gpsimd.memset(jg[:], 0.0)
            nc.gpsimd.tensor_scalar(jg[:], jg[:], 1.0, None,
                                    mybir.AluOpType.mult)
            nc.gpsimd.tensor_tensor(jg[:], jg[:], jg[:],
                                    mybir.AluOpType.mult)

            # ---- input / constant loads (sync = SP ring) ----
            cta = apool.tile([128, 288], f32)
            nc.sync.dma_start(cta[:], cta_t.ap())
            ctb = apool.tile([128, 315], bf16)
            nc.sync.dma_start(ctb[:], ctb_t.ap())
            dtw = cta[:, 0:F]
            bwsm = cta[:, F:F + K]
            vw = ctb[:, 0:F]
            vf0 = ctb[:, F:F + NB]
            # x windows in two chunks: x_all[i, b, d] = x[b*120+i, d]
            x_all = bpool.tile([128, NB, D], bf16)
            for c0, cn in ((0, CA), (CA, CB)):
                nc.sync.dma_start(
                    x_all[:, c0:c0 + cn, :],
                    bass.AP(x_t, c0 * BLK * D,
                            [[D, 128], [BLK * D, cn], [1, D]]))

            # ---- alpha stage, fully per chunk ----
            cv = apool.tile([128, F], f32)
            lgn = apool.tile([128, F], f32)
            s = apool.tile([128, NB], f32)
            sa = apool.tile([128, NB], f32)
            r = apool.tile([128, NB], f32)
            ei = apool.tile([128, F], f32)
            e = apool.tile([128, F], f32)
            sc = apool.tile([128, F], f32)
            au = apool.tile([128, F], f32)
            af = apool.tile([128, F], bf16)

            for c0, cn, eng, wt in ((0, CA, nc.vector, wza_t),
                                    (CA, CB, nc.gpsimd, wzb_t)):
                f0, f1 = c0 * K, (c0 + cn) * K
                # cv = vf[n]*vf[n+p]; lgn = (dt[n+p] - BIG)*cv  (= -lg)
                eng.tensor_tensor(cv[:, f0:f1], vw[:, f0:f1],
                                  exp_nb_h(vf0, c0, cn),
                                  mybir.AluOpType.mult)
                if eng is nc.vector:
                    eng.scalar_tensor_tensor(lgn[:, f0:f1], dtw[:, f0:f1],
                                             BIG, cv[:, f0:f1],
                                             mybir.AluOpType.subtract,
                                             mybir.AluOpType.mult)
                else:
                    eng.tensor_scalar(lgn[:, f0:f1], dtw[:, f0:f1], BIG,
                                      None, mybir.AluOpType.subtract)
                    eng.tensor_tensor(lgn[:, f0:f1], lgn[:, f0:f1],
                                      cv[:, f0:f1], mybir.AluOpType.mult)
                # shift = tap-0 strip of lgn: dt is sorted ascending, so
                # tap 0 is the per-position min of the (negated) exponents;
                # masked lanes are 0 and tap0 <= -924 keeps them ~exp(0)=0.
                # ei = lgn_0 - lgn  (== lg - mx of the positive form)
                lgn0 = bass.AP(lgn.tensor, lgn.offset + c0 * K,
                               [lgn.ap[0], [K, cn], [0, K]])
                eng.tensor_tensor(ei[:, f0:f1], lgn0, lgn[:, f0:f1],
                                  mybir.AluOpType.subtract)
                nc.scalar.activation(e[:, f0:f1], ei[:, f0:f1],
                                     mybir.ActivationFunctionType.Exp)
                nc.vector.tensor_reduce(s[:, c0:c0 + cn], pb_h(e, c0, cn),
                                        mybir.AxisListType.X,
                                        mybir.AluOpType.add)
                # scale-invariant blend: au = (e + s*c) * cv (c = bwsm row)
                eng.tensor_tensor(sc[:, f0:f1], exp_nb_h(s[:, :], c0, cn),
                                  exp_k_h(bwsm, cn), mybir.AluOpType.mult)
                eng.tensor_tensor(au[:, f0:f1], sc[:, f0:f1], e[:, f0:f1],
                                  mybir.AluOpType.add)
                # af = au*cv directly (bf16); the row normalization
                # r = vf0/max(sum af, eps) depends only on the OUTPUT row,
                # so it is applied as a per-partition scale at PSUM
                # eviction instead of gating the skew.
                eng.tensor_tensor(af[:, f0:f1], au[:, f0:f1], cv[:, f0:f1],
                                  mybir.AluOpType.mult)
                # skewed W write for this chunk (SP ring):
                # W^T[b][m, m+p] = af[m, p, b]
                nc.sync.dma_start(
                    bass.AP(wt, 0, [[KW + 1, BLK], [WBLK, cn], [1, K]]),
                    bass.AP(af.tensor, af.offset + c0 * K,
                            [af.ap[0], [K, cn], [1, K]])[:BLK, :, :])

            for c0, cn in ((0, CA), (CA, CB)):
                nc.vector.tensor_reduce(sa[:, c0:c0 + cn], pb_h(af, c0, cn),
                                        mybir.AxisListType.X,
                                        mybir.AluOpType.add)
            nc.vector.tensor_scalar(sa[:], sa[:], 1e-8, None,
                                    mybir.AluOpType.max)
            nc.vector.reciprocal(r[:], sa[:])
            nc.vector.tensor_tensor(r[:], r[:], vf0[:, :],
                                    mybir.AluOpType.mult)

            # ---- XBAR-transposed loads (Act ring, serialized) ----
            w_all = bpool.tile([128, NB, KW], bf16)
            nc.scalar.dma_start(
                w_all[:, 0:CA, :],
                bass.AP(wza_t, 0, [[KW, CA * KW], [1, KW]]),
                transpose=True)
            js = apool.tile([1, 1], f32)
            nc.scalar.activation(js[:1, :1], jv[:1, :1],
                                 mybir.ActivationFunctionType.Copy)
            nc.scalar.dma_start(
                w_all[:, CA:NB, :],
                bass.AP(wzb_t, 0, [[KW, CB * KW], [1, KW]]),
                transpose=True)

            # ---- per-block banded matmul; paired evict; group stores ----
            for g0, gb, seng in ((0, 8, nc.sync), (8, 8, nc.scalar),
                                 (16, 8, nc.sync), (24, 8, nc.scalar),
                                 (32, 2, nc.scalar)):
                out_g = opool.tile([128, 9, D], bf16, tag="og")
                for j in range(0, gb, 2):
                    b = g0 + j
                    npair = min(2, gb - j)
                    pt = psB.tile([KW, 2 * D], f32, tag="mm")
                    for q in range(npair):
                        nc.tensor.matmul(pt[:, q * D:(q + 1) * D],
                                         w_all[:, b + q, :],
                                         x_all[:, b + q, :])
                    for q in range(npair):
                        bq = b + q
                        if bq % 2 == 1:
                            nc.scalar.activation(
                                out_g[:BLK, j + q, :],
                                pt[:BLK, q * D:(q + 1) * D],
                                mybir.ActivationFunctionType.Copy,
                                scale=r[:BLK, bq:bq + 1])
                        else:
                            nc.vector.tensor_scalar(
                                out_g[:BLK, j + q, :],
                                pt[:BLK, q * D:(q + 1) * D],
                                r[:BLK, bq:bq + 1], None,
                                mybir.AluOpType.mult)
                seng.dma_start(
                    bass.AP(out_t, g0 * BLK * D,
                            [[D, BLK], [BLK * D, gb], [1, D]]),
                    out_g[:BLK, :gb, :])
    nc.compile()
    return nc


def _get_nc():
    if "nc" not in _CACHE:
        _CACHE["nc"] = _build()
    return _CACHE["nc"]


def _make_in_maps(x, delta_times, valid_mask, w, beta):
    w64 = w.astype(np.float64)
    wsm = np.exp(w64 - w64.max())
    wsm /= wsm.sum()
    b = 1.0 / (1.0 + np.exp(-float(beta[0])))
    bwsm = np.tile((b / (1.0 - b) * wsm)[None, :], (128, 1)).astype(np.float32)
    wza = np.zeros(CA * WBLK, np.float32).astype(ml_dtypes.bfloat16)
    wzb = np.zeros(CB * WBLK, np.float32).astype(ml_dtypes.bfloat16)
    # im2col window index: [m, b, p] -> b*120 + m + p
    widx = (np.arange(128)[:, None, None] + BLK * np.arange(NB)[None, :, None]
            + np.arange(K)[None, None, :])
    vidx = np.arange(128)[:, None] + BLK * np.arange(NB)[None, :]

    in_maps = []
    for i in range(B):
        xp = np.zeros((NPAD, D), np.float32)
        xp[:N] = x[i]
        dtp = np.zeros(NPAD, np.float32)
        dtp[:N] = delta_times[i]
        vfp = np.zeros(NPAD, np.float32)
        vfp[:N] = valid_mask[i].astype(np.float32)
        cta = np.zeros((128, 288), np.float32)
        cta[:, 0:F] = dtp[widx].reshape(128, F)
        cta[:, F:F + K] = bwsm
        ctb = np.zeros((128, 315), np.float32)
        ctb[:, 0:F] = vfp[widx].reshape(128, F)
        ctb[:, F:F + NB] = vfp[vidx]
        in_maps.append({
            "x": xp.astype(ml_dtypes.bfloat16),
            "cta": cta,
            "ctb": ctb.astype(ml_dtypes.bfloat16),
            "wza": wza,
            "wzb": wzb,
        })
    return in_maps


def _execute(in_maps, trace=False, **kw):
    nc = _get_nc()
    return run_bass_kernel_spmd(nc, in_maps, core_ids=list(range(B)),
                                trace=trace, **kw)


def kernel(x, delta_times, valid_mask, w, beta):
    in_maps = _make_in_maps(x, delta_times, valid_mask, w, beta)
    kr = _execute(in_maps, trace=False)
    outs = [kr.results[i]["out"][:N].astype(np.float32) for i in range(B)]
    return np.stack(outs, axis=0)
